# revision 3
# baseline (speedup 1.0000x reference)
"""Bi-Real Net binary conv2d (3x3, pad 1, stride 1) for Trainium2, 8 NeuronCores.

Math (forward values of the reference):
    xb = sign(x)                      in {-1, 0, +1}
    scale[o] = mean_{i,kh,kw} |w[o,i,kh,kw]|
    wb = scale[o] * sign(w)
    y = conv2d_NCHW(xb, wb, pad=1)

Kernel strategy:
    - Data-parallel over batch: 32 images -> 4 per core on 8 cores.
    - Per image: DMA [128, 112*112] f32 -> SBUF, ACT Sign -> zero-padded
      bf16 buffer [128, 114, 114].
    - Conv as 9 accumulated matmuls per 4-output-row chunk:
      psum[o, 4x112] += signW_tap[i, o].T @ xpad[i, rows+kh, kw:kw+112].
      Products are +-1 in bf16 (exact); PSUM accumulates exact integers.
    - PSUM evacuation on DVE multiplies by per-channel scale[o] (fp32).
    - Outputs staged in SBUF (16 rows) and DMA'd out in ~0.9 MB chunks.
"""

import sys

sys.path.insert(0, "/opt/trn_rl_repo")

import numpy as np

import concourse.bacc as bacc
import concourse.bass as bass
import concourse.mybir as mybir
import concourse.tile as tile
from concourse.bass_utils import run_bass_kernel_spmd
from concourse.masks import make_identity

N_CORES = 8
B, C, H, W = 32, 128, 112, 112
BL = B // N_CORES  # images per core
HP = H + 2  # padded height/width (114)
TAPS = [(kh, kw) for kh in range(3) for kw in range(3)]

F32 = mybir.dt.float32
BF16 = mybir.dt.bfloat16

N_ROWCHUNK = 4  # output rows per PSUM accumulation group (448 fp32 <= one bank)
N_STAGEROWS = 16  # output rows per SBUF->DRAM store
N_LOADROWS = 56  # input rows per DRAM->SBUF load


RP = 128  # fp8 padded-row pitch; 128 makes the DoubleRow mid-dim step %16==0

VARIANT = "fp8dr"  # "bf16" | "fp8dr"


def build_nc(variant=None):
    variant = variant or VARIANT
    fp8 = variant == "fp8dr"
    FP8 = mybir.dt.float8e4
    act_dt = FP8 if fp8 else BF16
    pitch = RP if fp8 else HP

    nc = bacc.Bacc(
        "TRN2", target_bir_lowering=False, debug=False, num_devices=N_CORES
    )
    x = nc.declare_dram_parameter("x", [BL, C, H, W], F32, isOutput=False)
    w = nc.declare_dram_parameter("weight", [C, C, 3, 3], F32, isOutput=False)
    y = nc.declare_dram_parameter("y", [BL, C, H, W], F32, isOutput=True)

    with tile.TileContext(nc) as tc:
        with (
            tc.tile_pool(name="consts", bufs=1) as consts,
            tc.tile_pool(name="psum", bufs=1, space="PSUM") as psum_pool,
        ):
            # ---- weight prep: scale[o] and transposed sign-weight tiles ----
            # bf16:  lhsT[i, tap, o] for the 9 taps
            # fp8dr: wdr[i, kw, j, o] pairs taps (kh=0,kw),(kh=1,kw); w2[i, kw, o]
            #        holds the kh=2 row
            if fp8:
                wdr = consts.tile([C, 3, 2, C], FP8)
                w2 = consts.tile([C, 3, C], FP8)
            else:
                lhsT = consts.tile([C, 9, C], BF16)  # [i, tap, o]
            scale = consts.tile([C, 1], F32)
            identity = consts.tile([C, C], BF16)
            make_identity(nc, identity)
            with tc.tile_pool(name="wprep", bufs=1) as wp:
                wf = wp.tile([C, C, 3, 3], F32)
                nc.sync.dma_start(wf[:, :, :, :], w[:, :, :, :])
                wabs = wp.tile([C, C, 3, 3], F32)
                ssum = wp.tile([C, 1], F32)
                nc.scalar.activation(
                    wabs[:, :, :, :],
                    wf[:, :, :, :],
                    mybir.ActivationFunctionType.Abs,
                    accum_out=ssum[:, :],
                )
                nc.scalar.mul(scale[:, :], ssum[:, :], 1.0 / (C * 9))
                wsign = wp.tile([C, C, 3, 3], BF16)
                nc.scalar.sign(wsign[:, :, :, :], wf[:, :, :, :])
                for t, (kh, kw) in enumerate(TAPS):
                    pst = psum_pool.tile([C, C], BF16, tag="pst", bufs=2)
                    nc.tensor.transpose(pst[:, :], wsign[:, :, kh, kw], identity[:, :])
                    if fp8:
                        dst = wdr[:, kw, kh, :] if kh < 2 else w2[:, kw, :]
                    else:
                        dst = lhsT[:, t, :]
                    nc.scalar.copy(dst, pst[:, :])

            # ---- main loop over local images ----
            with (
                tc.tile_pool(name="raw", bufs=2) as raw_pool,
                tc.tile_pool(name="xpad", bufs=2) as xpad_pool,
                tc.tile_pool(name="stage", bufs=3) as stage_pool,
            ):
                for n in range(BL):
                    xim = x[n]  # [C, H, W]
                    yim = y[n]
                    xpad = xpad_pool.tile([C, HP, pitch], act_dt, tag="xpad")
                    # zero the pad border (interior is fully overwritten below;
                    # cols >= 114 of the fp8 pitch-128 rows are never read)
                    nc.gpsimd.memset(xpad[:, 0, 0:HP], 0.0)
                    nc.gpsimd.memset(xpad[:, HP - 1, 0:HP], 0.0)
                    nc.gpsimd.memset(xpad[:, :, 0], 0.0)
                    nc.gpsimd.memset(xpad[:, :, HP - 1], 0.0)
                    for r0 in range(0, H, N_LOADROWS):
                        raw = raw_pool.tile([C, N_LOADROWS, W], F32, tag="raw")
                        nc.sync.dma_start(
                            raw[:, :, :], xim[:, r0 : r0 + N_LOADROWS, :]
                        )
                        # binarize in 28-row pieces for pipelining
                        step = N_LOADROWS // 2
                        for a in range(0, N_LOADROWS, step):
                            nc.scalar.sign(
                                xpad[:, r0 + a + 1 : r0 + a + step + 1, 1 : 1 + W],
                                raw[:, a : a + step, :],
                            )
                    for s0 in range(0, H, N_STAGEROWS):
                        stage = stage_pool.tile([C, N_STAGEROWS, W], F32, tag="stage")
                        for j in range(0, N_STAGEROWS, N_ROWCHUNK):
                            h0 = s0 + j
                            ps = psum_pool.tile(
                                [C, N_ROWCHUNK, W], F32, tag="ps", bufs=6
                            )
                            if fp8:
                                for kw in range(3):
                                    # taps (0,kw)+(1,kw) fused: K=256 DoubleRow
                                    base = xpad[:, h0, kw]
                                    rhs = bass.AP(
                                        tensor=base.tensor,
                                        offset=base.offset,
                                        ap=[
                                            base.ap[0],
                                            [pitch, 2],
                                            [pitch, N_ROWCHUNK],
                                            [1, W],
                                        ],
                                    )
                                    nc.tensor.matmul(
                                        ps[:, :, :],
                                        wdr[:, kw, :, :],
                                        rhs,
                                        start=(kw == 0),
                                        stop=False,
                                        perf_mode=mybir.MatmulPerfMode.DoubleRow,
                                    )
                                for kw in range(3):
                                    # tap (2,kw)
                                    nc.tensor.matmul(
                                        ps[:, :, :],
                                        w2[:, kw, :],
                                        xpad[
                                            :,
                                            h0 + 2 : h0 + 2 + N_ROWCHUNK,
                                            kw : kw + W,
                                        ],
                                        start=False,
                                        stop=(kw == 2),
                                    )
                            else:
                                for t, (kh, kw) in enumerate(TAPS):
                                    nc.tensor.matmul(
                                        ps[:, :, :],
                                        lhsT[:, t, :],
                                        xpad[
                                            :,
                                            h0 + kh : h0 + kh + N_ROWCHUNK,
                                            kw : kw + W,
                                        ],
                                        start=(t == 0),
                                        stop=(t == len(TAPS) - 1),
                                    )
                            nc.vector.tensor_scalar_mul(
                                stage[:, j : j + N_ROWCHUNK, :], ps[:, :, :], scale[:, :]
                            )
                        nc.gpsimd.dma_start(
                            yim[:, s0 : s0 + N_STAGEROWS, :], stage[:, :, :]
                        )

    nc.compile()
    return nc


_NC_CACHE = {}


def _get_nc(variant=None):
    variant = variant or VARIANT
    if variant not in _NC_CACHE:
        _NC_CACHE[variant] = build_nc(variant)
    return _NC_CACHE[variant]


def kernel(
    x: np.ndarray,
    weight: np.ndarray,
    _trace: bool = False,
    _variant: str | None = None,
    **_kw,
):
    assert x.shape == (B, C, H, W) and weight.shape == (C, C, 3, 3)
    nc = _get_nc(_variant)
    xs = np.ascontiguousarray(x, dtype=np.float32)
    wgt = np.ascontiguousarray(weight, dtype=np.float32)
    in_maps = [
        {"x": xs[i * BL : (i + 1) * BL], "weight": wgt} for i in range(N_CORES)
    ]
    res = run_bass_kernel_spmd(
        nc, in_maps, core_ids=list(range(N_CORES)), trace=_trace
    )
    out = np.concatenate([res.results[i]["y"] for i in range(N_CORES)], axis=0)
    if _trace:
        kernel.last_results = res
    return out


# revision 5
# speedup vs baseline: 1.1574x; 1.1574x over previous
"""Bi-Real Net binary conv2d (3x3, pad 1, stride 1) for Trainium2, 8 NeuronCores.

Math (forward values of the reference):
    xb = sign(x)                      in {-1, 0, +1}
    scale[o] = mean_{i,kh,kw} |w[o,i,kh,kw]|
    wb = scale[o] * sign(w)
    y = conv2d_NCHW(xb, wb, pad=1)

Kernel strategy:
    - Data-parallel over batch: 32 images -> 4 per core on 8 cores.
    - Per image: DMA [128, 112*112] f32 -> SBUF, ACT Sign -> zero-padded
      bf16 buffer [128, 114, 114].
    - Conv as 9 accumulated matmuls per 4-output-row chunk:
      psum[o, 4x112] += signW_tap[i, o].T @ xpad[i, rows+kh, kw:kw+112].
      Products are +-1 in bf16 (exact); PSUM accumulates exact integers.
    - PSUM evacuation on DVE multiplies by per-channel scale[o] (fp32).
    - Outputs staged in SBUF (16 rows) and DMA'd out in ~0.9 MB chunks.
"""

import sys

sys.path.insert(0, "/opt/trn_rl_repo")

import numpy as np

import concourse.bacc as bacc
import concourse.bass as bass
import concourse.mybir as mybir
import concourse.tile as tile
from concourse.bass_utils import run_bass_kernel_spmd
from concourse.masks import make_identity

N_CORES = 8
B, C, H, W = 32, 128, 112, 112
BL = B // N_CORES  # images per core
HP = H + 2  # padded height/width (114)
TAPS = [(kh, kw) for kh in range(3) for kw in range(3)]

F32 = mybir.dt.float32
BF16 = mybir.dt.bfloat16

N_ROWCHUNK = 4  # output rows per PSUM accumulation group (448 fp32 <= one bank)
N_STAGEROWS = 16  # output rows per SBUF->DRAM store
N_LOADROWS = 56  # input rows per DRAM->SBUF load


RP = 128  # fp8 padded-row pitch; 128 makes the DoubleRow mid-dim step %16==0

VARIANT = "fp8dr"  # "bf16" | "fp8dr"


def build_nc(variant=None):
    variant = variant or VARIANT
    fp8 = variant == "fp8dr"
    FP8 = mybir.dt.float8e4
    act_dt = FP8 if fp8 else BF16
    pitch = RP if fp8 else HP

    nc = bacc.Bacc(
        "TRN2", target_bir_lowering=False, debug=False, num_devices=N_CORES
    )
    x = nc.declare_dram_parameter("x", [BL, C, H, W], F32, isOutput=False)
    w = nc.declare_dram_parameter("weight", [C, C, 3, 3], F32, isOutput=False)
    y = nc.declare_dram_parameter("y", [BL, C, H, W], F32, isOutput=True)

    with tile.TileContext(nc) as tc:
        with (
            tc.tile_pool(name="consts", bufs=1) as consts,
            tc.tile_pool(name="psum", bufs=1, space="PSUM") as psum_pool,
        ):
            # ---- weight prep: scale[o] and transposed sign-weight tiles ----
            # bf16:  lhsT[i, tap, o] for the 9 taps
            # fp8dr: wdr[i, kw, j, o] pairs taps (kh=0,kw),(kh=1,kw); w2[i, kw, o]
            #        holds the kh=2 row
            if fp8:
                wdr = consts.tile([C, 3, 2, C], FP8)
                w2 = consts.tile([C, 3, C], FP8)
            else:
                lhsT = consts.tile([C, 9, C], BF16)  # [i, tap, o]
            scale = consts.tile([C, 1], F32)
            identity = consts.tile([C, C], BF16)
            make_identity(nc, identity)
            with tc.tile_pool(name="wprep", bufs=1) as wp:
                wf = wp.tile([C, C, 3, 3], F32)
                nc.sync.dma_start(wf[:, :, :, :], w[:, :, :, :])
                wabs = wp.tile([C, C, 3, 3], F32)
                ssum = wp.tile([C, 1], F32)
                nc.scalar.activation(
                    wabs[:, :, :, :],
                    wf[:, :, :, :],
                    mybir.ActivationFunctionType.Abs,
                    accum_out=ssum[:, :],
                )
                nc.scalar.mul(scale[:, :], ssum[:, :], 1.0 / (C * 9))
                wsign = wp.tile([C, C, 3, 3], BF16)
                nc.scalar.sign(wsign[:, :, :, :], wf[:, :, :, :])
                for t, (kh, kw) in enumerate(TAPS):
                    pst = psum_pool.tile([C, C], BF16, tag="pst", bufs=2)
                    nc.tensor.transpose(pst[:, :], wsign[:, :, kh, kw], identity[:, :])
                    if fp8:
                        dst = wdr[:, kw, kh, :] if kh < 2 else w2[:, kw, :]
                    else:
                        dst = lhsT[:, t, :]
                    nc.scalar.copy(dst, pst[:, :])

            # ---- main loop over local images ----
            with (
                tc.tile_pool(name="raw", bufs=2) as raw_pool,
                tc.tile_pool(name="xpad", bufs=2) as xpad_pool,
                tc.tile_pool(name="stage", bufs=3) as stage_pool,
            ):
                for n in range(BL):
                    xim = x[n]  # [C, H, W]
                    yim = y[n]
                    # fp8dr reads whole pitch-128 rows (N=512 contiguous spans);
                    # one extra dummy row absorbs the last chunk's 2-element
                    # overrun, and every non-interior cell is zeroed.
                    nrows = HP + 1 if fp8 else HP
                    xpad = xpad_pool.tile([C, nrows, pitch], act_dt, tag="xpad")
                    if fp8:
                        nc.gpsimd.memset(xpad[:, 0, :], 0.0)
                        nc.gpsimd.memset(xpad[:, HP - 1 :, :], 0.0)
                        nc.gpsimd.memset(xpad[:, :, 0], 0.0)
                        nc.gpsimd.memset(xpad[:, :, W + 1 : pitch], 0.0)
                    else:
                        nc.gpsimd.memset(xpad[:, 0, :], 0.0)
                        nc.gpsimd.memset(xpad[:, HP - 1, :], 0.0)
                        nc.gpsimd.memset(xpad[:, :, 0], 0.0)
                        nc.gpsimd.memset(xpad[:, :, HP - 1], 0.0)
                    for r0 in range(0, H, N_LOADROWS):
                        raw = raw_pool.tile([C, N_LOADROWS, W], F32, tag="raw")
                        nc.sync.dma_start(
                            raw[:, :, :], xim[:, r0 : r0 + N_LOADROWS, :]
                        )
                        # binarize in 28-row pieces for pipelining
                        step = N_LOADROWS // 2
                        for a in range(0, N_LOADROWS, step):
                            nc.scalar.sign(
                                xpad[:, r0 + a + 1 : r0 + a + step + 1, 1 : 1 + W],
                                raw[:, a : a + step, :],
                            )
                    for s0 in range(0, H, N_STAGEROWS):
                        stage = stage_pool.tile([C, N_STAGEROWS, W], F32, tag="stage")
                        for j in range(0, N_STAGEROWS, N_ROWCHUNK):
                            h0 = s0 + j
                            if fp8:
                                # full-pitch output rows: N = 4*128 = 512 fp32
                                # (one PSUM bank); cols >= 112 of each row are
                                # garbage and skipped at evacuation
                                NF = N_ROWCHUNK * pitch
                                ps = psum_pool.tile([C, NF], F32, tag="ps", bufs=6)
                                for kw in range(3):
                                    # taps (0,kw)+(1,kw) fused: K=256 DoubleRow
                                    base = xpad[:, h0, kw]
                                    rhs = bass.AP(
                                        tensor=base.tensor,
                                        offset=base.offset,
                                        ap=[base.ap[0], [pitch, 2], [1, NF]],
                                    )
                                    nc.tensor.matmul(
                                        ps[:, :],
                                        wdr[:, kw, :, :],
                                        rhs,
                                        start=(kw == 0),
                                        stop=False,
                                        perf_mode=mybir.MatmulPerfMode.DoubleRow,
                                    )
                                for kw in range(3):
                                    # tap (2,kw)
                                    base = xpad[:, h0 + 2, kw]
                                    rhs = bass.AP(
                                        tensor=base.tensor,
                                        offset=base.offset,
                                        ap=[base.ap[0], [1, NF]],
                                    )
                                    nc.tensor.matmul(
                                        ps[:, :],
                                        w2[:, kw, :],
                                        rhs,
                                        start=False,
                                        stop=(kw == 2),
                                    )
                                ps_rows = ps.rearrange(
                                    "p (a b) -> p a b", b=pitch
                                )[:, :, 0:W]
                            else:
                                ps = psum_pool.tile(
                                    [C, N_ROWCHUNK, W], F32, tag="ps", bufs=6
                                )
                                for t, (kh, kw) in enumerate(TAPS):
                                    nc.tensor.matmul(
                                        ps[:, :, :],
                                        lhsT[:, t, :],
                                        xpad[
                                            :,
                                            h0 + kh : h0 + kh + N_ROWCHUNK,
                                            kw : kw + W,
                                        ],
                                        start=(t == 0),
                                        stop=(t == len(TAPS) - 1),
                                    )
                                ps_rows = ps[:, :, :]
                            nc.vector.tensor_scalar_mul(
                                stage[:, j : j + N_ROWCHUNK, :], ps_rows, scale[:, :]
                            )
                        nc.gpsimd.dma_start(
                            yim[:, s0 : s0 + N_STAGEROWS, :], stage[:, :, :]
                        )

    nc.compile()
    return nc


_NC_CACHE = {}


def _get_nc(variant=None):
    variant = variant or VARIANT
    if variant not in _NC_CACHE:
        _NC_CACHE[variant] = build_nc(variant)
    return _NC_CACHE[variant]


def kernel(
    x: np.ndarray,
    weight: np.ndarray,
    _trace: bool = False,
    _variant: str | None = None,
    **_kw,
):
    assert x.shape == (B, C, H, W) and weight.shape == (C, C, 3, 3)
    nc = _get_nc(_variant)
    xs = np.ascontiguousarray(x, dtype=np.float32)
    wgt = np.ascontiguousarray(weight, dtype=np.float32)
    in_maps = [
        {"x": xs[i * BL : (i + 1) * BL], "weight": wgt} for i in range(N_CORES)
    ]
    res = run_bass_kernel_spmd(
        nc, in_maps, core_ids=list(range(N_CORES)), trace=_trace
    )
    out = np.concatenate([res.results[i]["y"] for i in range(N_CORES)], axis=0)
    if _trace:
        kernel.last_results = res
    return out


# revision 11
# speedup vs baseline: 1.2561x; 1.0852x over previous
"""Bi-Real Net binary conv2d (3x3, pad 1, stride 1) for Trainium2, 8 NeuronCores.

Math (forward values of the reference):
    xb = sign(x)                      in {-1, 0, +1}
    scale[o] = mean_{i,kh,kw} |w[o,i,kh,kw]|
    wb = scale[o] * sign(w)
    y = conv2d_NCHW(xb, wb, pad=1)

Kernel strategy:
    - Data-parallel over batch: 32 images -> 4 per core on 8 cores.
    - Per image: DMA [128, 112*112] f32 -> SBUF, ACT Sign -> zero-padded
      bf16 buffer [128, 114, 114].
    - Conv as 9 accumulated matmuls per 4-output-row chunk:
      psum[o, 4x112] += signW_tap[i, o].T @ xpad[i, rows+kh, kw:kw+112].
      Products are +-1 in bf16 (exact); PSUM accumulates exact integers.
    - PSUM evacuation on DVE multiplies by per-channel scale[o] (fp32).
    - Outputs staged in SBUF (16 rows) and DMA'd out in ~0.9 MB chunks.
"""

import sys

sys.path.insert(0, "/opt/trn_rl_repo")

import numpy as np

import concourse.bacc as bacc
import concourse.bass as bass
import concourse.mybir as mybir
import concourse.tile as tile
from concourse.bass_utils import run_bass_kernel_spmd
from concourse.masks import make_identity

N_CORES = 8
B, C, H, W = 32, 128, 112, 112
BL = B // N_CORES  # images per core
HP = H + 2  # padded height/width (114)
TAPS = [(kh, kw) for kh in range(3) for kw in range(3)]

F32 = mybir.dt.float32
BF16 = mybir.dt.bfloat16

N_ROWCHUNK = 4  # output rows per PSUM accumulation group (<= one 2KB bank)
N_STAGEROWS = 28  # output rows per SBUF->DRAM store (must divide 112)
N_LOADROWS = 28  # input rows per DRAM->SBUF load
N_SIGNROWS = 14  # input rows per ACT Sign instruction


RP = 128  # fp8 padded-row pitch; 128 makes the DoubleRow mid-dim step %16==0

VARIANT = "fp8dr"  # "bf16" | "fp8dr"


def build_nc(variant=None):
    variant = variant or VARIANT
    fp8 = variant == "fp8dr"
    FP8 = mybir.dt.float8e4
    act_dt = FP8 if fp8 else BF16
    pitch = RP if fp8 else HP

    nc = bacc.Bacc(
        "TRN2", target_bir_lowering=False, debug=False, num_devices=N_CORES
    )
    x = nc.declare_dram_parameter("x", [BL, C, H, W], F32, isOutput=False)
    w = nc.declare_dram_parameter("weight", [C, C, 3, 3], F32, isOutput=False)
    y = nc.declare_dram_parameter("y", [BL, C, H, W], F32, isOutput=True)

    with tile.TileContext(nc) as tc:
        with (
            tc.tile_pool(name="consts", bufs=1) as consts,
            tc.tile_pool(name="psum", bufs=1, space="PSUM") as psum_pool,
        ):
            # ---- weight prep: scale[o] and transposed sign-weight tiles ----
            # bf16:  lhsT[i, tap, o] for the 9 taps
            # fp8dr: wdr[i, kw, j, o] pairs taps (kh=0,kw),(kh=1,kw); w2[i, kw, o]
            #        holds the kh=2 row
            if fp8:
                wdr = consts.tile([C, 3, 2, C], FP8)
                w2 = consts.tile([C, 3, C], FP8)
            else:
                lhsT = consts.tile([C, 9, C], BF16)  # [i, tap, o]
            scale = consts.tile([C, 1], F32)
            identity = consts.tile([C, C], BF16)
            make_identity(nc, identity)
            with tc.tile_pool(name="wprep", bufs=1) as wp:
                wf = wp.tile([C, C, 3, 3], F32)
                nc.sync.dma_start(wf[:, :, :, :], w[:, :, :, :])
                wabs = wp.tile([C, C, 3, 3], F32)
                ssum = wp.tile([C, 1], F32)
                nc.scalar.activation(
                    wabs[:, :, :, :],
                    wf[:, :, :, :],
                    mybir.ActivationFunctionType.Abs,
                    accum_out=ssum[:, :],
                )
                nc.scalar.mul(scale[:, :], ssum[:, :], 1.0 / (C * 9))
                wsign = wp.tile([C, C, 3, 3], BF16)
                nc.scalar.sign(wsign[:, :, :, :], wf[:, :, :, :])
                for t, (kh, kw) in enumerate(TAPS):
                    pst = psum_pool.tile([C, C], BF16, tag="pst", bufs=2)
                    nc.tensor.transpose(pst[:, :], wsign[:, :, kh, kw], identity[:, :])
                    if fp8:
                        dst = wdr[:, kw, kh, :] if kh < 2 else w2[:, kw, :]
                    else:
                        dst = lhsT[:, t, :]
                    # DVE, not ACT: keeps ACT free for the first image's Sign
                    nc.vector.tensor_copy(dst, pst[:, :])

            # ---- main loop over local images ----
            with (
                tc.tile_pool(name="raw", bufs=2) as raw_pool,
                tc.tile_pool(name="xpad", bufs=1) as xpad_pool,
                tc.tile_pool(name="stage", bufs=3) as stage_pool,
            ):
                # Two persistent padded buffers, manually double-buffered
                # across images. Borders are zeroed ONCE here (the interior is
                # rewritten per image, borders stay zero), so image-boundary
                # matmuls never wait on memsets queued behind output DMAs.
                # fp8dr reads whole pitch-128 rows (N=512 contiguous spans);
                # one extra dummy row absorbs the last chunk's 2-element
                # overrun, and every non-interior cell is zeroed.
                nrows = HP + 1 if fp8 else HP
                xpads = []
                for k in range(2):
                    xp = xpad_pool.tile(
                        [C, nrows, pitch], act_dt, tag=f"xpad{k}", name=f"xpad{k}"
                    )
                    nc.gpsimd.memset(xp[:, 0, :], 0.0)
                    if fp8:
                        nc.gpsimd.memset(xp[:, HP - 1 :, :], 0.0)
                        nc.gpsimd.memset(xp[:, :, W + 1 : pitch], 0.0)
                    else:
                        nc.gpsimd.memset(xp[:, HP - 1, :], 0.0)
                        nc.gpsimd.memset(xp[:, :, HP - 1], 0.0)
                    nc.gpsimd.memset(xp[:, :, 0], 0.0)
                    xpads.append(xp)
                for n in range(BL):
                    xim = x[n]  # [C, H, W]
                    yim = y[n]
                    xpad = xpads[n % 2]
                    for r0 in range(0, H, N_LOADROWS):
                        raw = raw_pool.tile(
                            [C, N_LOADROWS, W], F32, tag="raw", bufs=4
                        )
                        nc.sync.dma_start(
                            raw[:, :, :], xim[:, r0 : r0 + N_LOADROWS, :]
                        )
                        for a in range(0, N_LOADROWS, N_SIGNROWS):
                            nc.scalar.sign(
                                xpad[
                                    :, r0 + a + 1 : r0 + a + N_SIGNROWS + 1, 1 : 1 + W
                                ],
                                raw[:, a : a + N_SIGNROWS, :],
                            )
                    for s0 in range(0, H, N_STAGEROWS):
                        stage = stage_pool.tile([C, N_STAGEROWS, W], F32, tag="stage")
                        for j in range(0, N_STAGEROWS, N_ROWCHUNK):
                            h0 = s0 + j
                            if fp8:
                                # full-pitch output rows: N = 4*128 = 512 fp32
                                # (one PSUM bank); cols >= 112 of each row are
                                # garbage and skipped at evacuation
                                NF = N_ROWCHUNK * pitch
                                ps = psum_pool.tile([C, NF], F32, tag="ps", bufs=6)
                                for kw in range(3):
                                    # taps (0,kw)+(1,kw) fused: K=256 DoubleRow
                                    base = xpad[:, h0, kw]
                                    rhs = bass.AP(
                                        tensor=base.tensor,
                                        offset=base.offset,
                                        ap=[base.ap[0], [pitch, 2], [1, NF]],
                                    )
                                    nc.tensor.matmul(
                                        ps[:, :],
                                        wdr[:, kw, :, :],
                                        rhs,
                                        start=(kw == 0),
                                        stop=False,
                                        perf_mode=mybir.MatmulPerfMode.DoubleRow,
                                    )
                                for kw in range(3):
                                    # tap (2,kw)
                                    base = xpad[:, h0 + 2, kw]
                                    rhs = bass.AP(
                                        tensor=base.tensor,
                                        offset=base.offset,
                                        ap=[base.ap[0], [1, NF]],
                                    )
                                    nc.tensor.matmul(
                                        ps[:, :],
                                        w2[:, kw, :],
                                        rhs,
                                        start=False,
                                        stop=(kw == 2),
                                    )
                                ps_rows = ps.rearrange(
                                    "p (a b) -> p a b", b=pitch
                                )[:, :, 0:W]
                            else:
                                ps = psum_pool.tile(
                                    [C, N_ROWCHUNK, W], F32, tag="ps", bufs=6
                                )
                                for t, (kh, kw) in enumerate(TAPS):
                                    nc.tensor.matmul(
                                        ps[:, :, :],
                                        lhsT[:, t, :],
                                        xpad[
                                            :,
                                            h0 + kh : h0 + kh + N_ROWCHUNK,
                                            kw : kw + W,
                                        ],
                                        start=(t == 0),
                                        stop=(t == len(TAPS) - 1),
                                    )
                                ps_rows = ps[:, :, :]
                            nc.vector.tensor_scalar_mul(
                                stage[:, j : j + N_ROWCHUNK, :], ps_rows, scale[:, :]
                            )
                        nc.gpsimd.dma_start(
                            yim[:, s0 : s0 + N_STAGEROWS, :], stage[:, :, :]
                        )

    nc.compile()
    return nc


_NC_CACHE = {}


def _get_nc(variant=None):
    variant = variant or VARIANT
    if variant not in _NC_CACHE:
        _NC_CACHE[variant] = build_nc(variant)
    return _NC_CACHE[variant]


def kernel(
    x: np.ndarray,
    weight: np.ndarray,
    _trace: bool = False,
    _variant: str | None = None,
    **_kw,
):
    assert x.shape == (B, C, H, W) and weight.shape == (C, C, 3, 3)
    nc = _get_nc(_variant)
    xs = np.ascontiguousarray(x, dtype=np.float32)
    wgt = np.ascontiguousarray(weight, dtype=np.float32)
    in_maps = [
        {"x": xs[i * BL : (i + 1) * BL], "weight": wgt} for i in range(N_CORES)
    ]
    res = run_bass_kernel_spmd(
        nc, in_maps, core_ids=list(range(N_CORES)), trace=_trace
    )
    out = np.concatenate([res.results[i]["y"] for i in range(N_CORES)], axis=0)
    if _trace:
        kernel.last_results = res
    return out


# revision 19
# speedup vs baseline: 1.3467x; 1.0721x over previous
"""Bi-Real Net binary conv2d (3x3, pad 1, stride 1) for Trainium2, 8 NeuronCores.

Math (forward values of the reference):
    xb = sign(x)                      in {-1, 0, +1}
    scale[o] = mean_{i,kh,kw} |w[o,i,kh,kw]|
    wb = scale[o] * sign(w)
    y = conv2d_NCHW(xb, wb, pad=1)

Kernel strategy:
    - Data-parallel over batch: 32 images -> 4 per core on 8 cores.
    - Per image: DMA [128, 112*112] f32 -> SBUF, ACT Sign -> zero-padded
      bf16 buffer [128, 114, 114].
    - Conv as 9 accumulated matmuls per 4-output-row chunk:
      psum[o, 4x112] += signW_tap[i, o].T @ xpad[i, rows+kh, kw:kw+112].
      Products are +-1 in bf16 (exact); PSUM accumulates exact integers.
    - PSUM evacuation on DVE multiplies by per-channel scale[o] (fp32).
    - Outputs staged in SBUF (16 rows) and DMA'd out in ~0.9 MB chunks.
"""

import sys

sys.path.insert(0, "/opt/trn_rl_repo")

import numpy as np

import concourse.bacc as bacc
import concourse.bass as bass
import concourse.mybir as mybir
import concourse.tile as tile
from concourse.bass_utils import run_bass_kernel_spmd
from concourse.masks import make_identity

N_CORES = 8
B, C, H, W = 32, 128, 112, 112
BL = B // N_CORES  # images per core
HP = H + 2  # padded height/width (114)
TAPS = [(kh, kw) for kh in range(3) for kw in range(3)]

F32 = mybir.dt.float32
BF16 = mybir.dt.bfloat16

N_ROWCHUNK = 4  # output rows per PSUM accumulation group (<= one 2KB bank)
N_STAGEROWS = 28  # output rows per SBUF->DRAM store (must divide 112)
N_LOADROWS = 28  # input rows per DRAM->SBUF load
N_SIGNROWS = 14  # input rows per ACT Sign instruction


RP = 128  # fp8 padded-row pitch; 128 makes the DoubleRow mid-dim step %16==0

VARIANT = "fp8dr5"  # "bf16" | "fp8dr" | "fp8dr5"


def build_nc(variant=None):
    variant = variant or VARIANT
    fp8 = variant in ("fp8dr", "fp8dr5")
    # fp8dr5: a second, column-shifted plane P1[r,c] = P0[r,c+1] lets taps
    # (2,0)+(2,1) share one DoubleRow matmul (pair step = plane stride), so a
    # chunk needs 5 matmuls instead of 6.
    planes = variant == "fp8dr5"
    FP8 = mybir.dt.float8e4
    act_dt = FP8 if fp8 else BF16
    pitch = RP if fp8 else HP

    nc = bacc.Bacc(
        "TRN2", target_bir_lowering=False, debug=False, num_devices=N_CORES
    )
    x = nc.declare_dram_parameter("x", [BL, C, H, W], F32, isOutput=False)
    w = nc.declare_dram_parameter("weight", [C, C, 3, 3], F32, isOutput=False)
    y = nc.declare_dram_parameter("y", [BL, C, H, W], F32, isOutput=True)

    with tile.TileContext(nc) as tc:
        with (
            tc.tile_pool(name="consts", bufs=1) as consts,
            tc.tile_pool(name="psum", bufs=1, space="PSUM") as psum_pool,
        ):
            # ---- weight prep: scale[o] and transposed sign-weight tiles ----
            # bf16:  lhsT[i, tap, o] for the 9 taps
            # fp8dr: wdr[i, kw, j, o] pairs taps (kh=0,kw),(kh=1,kw); w2[i, kw, o]
            #        holds the kh=2 row
            if fp8:
                wdr = consts.tile([C, 3, 2, C], FP8)
                if planes:
                    wp2 = consts.tile([C, 2, C], FP8)  # taps (2,0),(2,1)
                    w22 = consts.tile([C, C], FP8)  # tap (2,2)
                else:
                    w2 = consts.tile([C, 3, C], FP8)
            else:
                lhsT = consts.tile([C, 9, C], BF16)  # [i, tap, o]
            scale = consts.tile([C, 1], F32)
            identity = consts.tile([C, C], BF16)
            make_identity(nc, identity)
            with tc.tile_pool(name="wprep", bufs=1) as wp:
                wf = wp.tile([C, C, 3, 3], F32)
                nc.sync.dma_start(wf[:, :, :, :], w[:, :, :, :])
                wabs = wp.tile([C, C, 3, 3], F32)
                ssum = wp.tile([C, 1], F32)
                nc.scalar.activation(
                    wabs[:, :, :, :],
                    wf[:, :, :, :],
                    mybir.ActivationFunctionType.Abs,
                    accum_out=ssum[:, :],
                )
                nc.scalar.mul(scale[:, :], ssum[:, :], 1.0 / (C * 9))
                wsign = wp.tile([C, C, 3, 3], BF16)
                nc.scalar.sign(wsign[:, :, :, :], wf[:, :, :, :])
                for t, (kh, kw) in enumerate(TAPS):
                    pst = psum_pool.tile([C, C], BF16, tag="pst", bufs=2)
                    nc.tensor.transpose(pst[:, :], wsign[:, :, kh, kw], identity[:, :])
                    if fp8 and planes:
                        if kh < 2:
                            dst = wdr[:, kw, kh, :]
                        elif kw < 2:
                            dst = wp2[:, kw, :]
                        else:
                            dst = w22[:, :]
                    elif fp8:
                        dst = wdr[:, kw, kh, :] if kh < 2 else w2[:, kw, :]
                    else:
                        dst = lhsT[:, t, :]
                    # DVE, not ACT: keeps ACT free for the first image's Sign
                    nc.vector.tensor_copy(dst, pst[:, :])

            # ---- main loop over local images ----
            with (
                tc.tile_pool(name="raw", bufs=2) as raw_pool,
                tc.tile_pool(name="xpad", bufs=1) as xpad_pool,
                tc.tile_pool(name="stage", bufs=3) as stage_pool,
            ):
                # Two persistent padded buffers, manually double-buffered
                # across images. Borders are zeroed ONCE here (the interior is
                # rewritten per image, borders stay zero), so image-boundary
                # matmuls never wait on memsets queued behind output DMAs.
                # fp8dr reads whole pitch-128 rows (N=512 contiguous spans);
                # one extra dummy row absorbs the last chunk's 2-element
                # overrun, and every non-interior cell is zeroed.
                nrows = HP + 1 if fp8 else HP
                nplanes = 2 if planes else 1
                xpads = []
                for k in range(2):
                    xp = xpad_pool.tile(
                        [C, nplanes, nrows, pitch],
                        act_dt,
                        tag=f"xpad{k}",
                        name=f"xpad{k}",
                    )
                    nc.gpsimd.memset(xp[:, 0, 0, :], 0.0)
                    if fp8:
                        nc.gpsimd.memset(xp[:, 0, HP - 1 :, :], 0.0)
                        nc.gpsimd.memset(xp[:, 0, :, W + 1 : pitch], 0.0)
                    else:
                        nc.gpsimd.memset(xp[:, 0, HP - 1, :], 0.0)
                        nc.gpsimd.memset(xp[:, 0, :, HP - 1], 0.0)
                    nc.gpsimd.memset(xp[:, 0, :, 0], 0.0)
                    if planes:
                        # P1[r,c] = P0[r,c+1]; interior cols 0..111 are written
                        # per image, everything else is static zero
                        nc.gpsimd.memset(xp[:, 1, 0:2, :], 0.0)
                        nc.gpsimd.memset(xp[:, 1, HP - 1 :, :], 0.0)
                        nc.gpsimd.memset(xp[:, 1, :, W:pitch], 0.0)
                    xpads.append(xp)
                for n in range(BL):
                    xim = x[n]  # [C, H, W]
                    yim = y[n]
                    xpad = xpads[n % 2]
                    for r0 in range(0, H, N_LOADROWS):
                        raw = raw_pool.tile(
                            [C, N_LOADROWS, W], F32, tag="raw", bufs=4
                        )
                        nc.sync.dma_start(
                            raw[:, :, :], xim[:, r0 : r0 + N_LOADROWS, :]
                        )
                        for a in range(0, N_LOADROWS, N_SIGNROWS):
                            rr = r0 + a + 1
                            nc.scalar.sign(
                                xpad[:, 0, rr : rr + N_SIGNROWS, 1 : 1 + W],
                                raw[:, a : a + N_SIGNROWS, :],
                            )
                            if planes:
                                nc.scalar.sign(
                                    xpad[:, 1, rr : rr + N_SIGNROWS, 0:W],
                                    raw[:, a : a + N_SIGNROWS, :],
                                )
                    for s0 in range(0, H, N_STAGEROWS):
                        stage = stage_pool.tile([C, N_STAGEROWS, W], F32, tag="stage")
                        for j in range(0, N_STAGEROWS, N_ROWCHUNK):
                            h0 = s0 + j
                            if fp8:
                                # full-pitch output rows: N = 4*128 = 512 fp32
                                # (one PSUM bank); cols >= 112 of each row are
                                # garbage and skipped at evacuation
                                NF = N_ROWCHUNK * pitch
                                ps = psum_pool.tile([C, NF], F32, tag="ps", bufs=6)
                                for kw in range(3):
                                    # taps (0,kw)+(1,kw) fused: K=256 DoubleRow
                                    base = xpad[:, 0, h0, kw]
                                    rhs = bass.AP(
                                        tensor=base.tensor,
                                        offset=base.offset,
                                        ap=[base.ap[0], [pitch, 2], [1, NF]],
                                    )
                                    nc.tensor.matmul(
                                        ps[:, :],
                                        wdr[:, kw, :, :],
                                        rhs,
                                        start=(kw == 0),
                                        stop=False,
                                        perf_mode=mybir.MatmulPerfMode.DoubleRow,
                                    )
                                if planes:
                                    # taps (2,0)+(2,1) fused across the P0/P1
                                    # planes (pair step = plane stride)
                                    base = xpad[:, 0, h0 + 2, 0]
                                    rhs = bass.AP(
                                        tensor=base.tensor,
                                        offset=base.offset,
                                        ap=[base.ap[0], [nrows * pitch, 2], [1, NF]],
                                    )
                                    nc.tensor.matmul(
                                        ps[:, :],
                                        wp2[:, :, :],
                                        rhs,
                                        start=False,
                                        stop=False,
                                        perf_mode=mybir.MatmulPerfMode.DoubleRow,
                                    )
                                    base = xpad[:, 0, h0 + 2, 2]
                                    rhs = bass.AP(
                                        tensor=base.tensor,
                                        offset=base.offset,
                                        ap=[base.ap[0], [1, NF]],
                                    )
                                    nc.tensor.matmul(
                                        ps[:, :],
                                        w22[:, :],
                                        rhs,
                                        start=False,
                                        stop=True,
                                    )
                                else:
                                    for kw in range(3):
                                        # tap (2,kw)
                                        base = xpad[:, 0, h0 + 2, kw]
                                        rhs = bass.AP(
                                            tensor=base.tensor,
                                            offset=base.offset,
                                            ap=[base.ap[0], [1, NF]],
                                        )
                                        nc.tensor.matmul(
                                            ps[:, :],
                                            w2[:, kw, :],
                                            rhs,
                                            start=False,
                                            stop=(kw == 2),
                                        )
                                ps_rows = ps.rearrange(
                                    "p (a b) -> p a b", b=pitch
                                )[:, :, 0:W]
                            else:
                                ps = psum_pool.tile(
                                    [C, N_ROWCHUNK, W], F32, tag="ps", bufs=6
                                )
                                for t, (kh, kw) in enumerate(TAPS):
                                    nc.tensor.matmul(
                                        ps[:, :, :],
                                        lhsT[:, t, :],
                                        xpad[
                                            :,
                                            0,
                                            h0 + kh : h0 + kh + N_ROWCHUNK,
                                            kw : kw + W,
                                        ],
                                        start=(t == 0),
                                        stop=(t == len(TAPS) - 1),
                                    )
                                ps_rows = ps[:, :, :]
                            nc.vector.tensor_scalar_mul(
                                stage[:, j : j + N_ROWCHUNK, :], ps_rows, scale[:, :]
                            )
                        nc.gpsimd.dma_start(
                            yim[:, s0 : s0 + N_STAGEROWS, :], stage[:, :, :]
                        )

    nc.compile()
    return nc


_NC_CACHE = {}


def _get_nc(variant=None):
    variant = variant or VARIANT
    if variant not in _NC_CACHE:
        _NC_CACHE[variant] = build_nc(variant)
    return _NC_CACHE[variant]


def kernel(
    x: np.ndarray,
    weight: np.ndarray,
    _trace: bool = False,
    _variant: str | None = None,
    **_kw,
):
    assert x.shape == (B, C, H, W) and weight.shape == (C, C, 3, 3)
    nc = _get_nc(_variant)
    xs = np.ascontiguousarray(x, dtype=np.float32)
    wgt = np.ascontiguousarray(weight, dtype=np.float32)
    in_maps = [
        {"x": xs[i * BL : (i + 1) * BL], "weight": wgt} for i in range(N_CORES)
    ]
    res = run_bass_kernel_spmd(
        nc, in_maps, core_ids=list(range(N_CORES)), trace=_trace
    )
    out = np.concatenate([res.results[i]["y"] for i in range(N_CORES)], axis=0)
    if _trace:
        kernel.last_results = res
    return out


# revision 30
# speedup vs baseline: 1.4408x; 1.0699x over previous
"""Bi-Real Net binary conv2d (3x3, pad 1, stride 1) for Trainium2, 8 NeuronCores.

Math (forward values of the reference):
    xb = sign(x)                      in {-1, 0, +1}
    scale[o] = mean_{i,kh,kw} |w[o,i,kh,kw]|
    wb = scale[o] * sign(w)
    y = conv2d_NCHW(xb, wb, pad=1)

Kernel strategy:
    - Data-parallel over batch: 32 images -> 4 per core on 8 cores.
    - Per image: DMA [128, 112*112] f32 -> SBUF, ACT Sign -> zero-padded
      bf16 buffer [128, 114, 114].
    - Conv as 9 accumulated matmuls per 4-output-row chunk:
      psum[o, 4x112] += signW_tap[i, o].T @ xpad[i, rows+kh, kw:kw+112].
      Products are +-1 in bf16 (exact); PSUM accumulates exact integers.
    - PSUM evacuation on DVE multiplies by per-channel scale[o] (fp32).
    - Outputs staged in SBUF (16 rows) and DMA'd out in ~0.9 MB chunks.
"""

import sys

sys.path.insert(0, "/opt/trn_rl_repo")

import numpy as np

import concourse.bacc as bacc
import concourse.bass as bass
import concourse.mybir as mybir
import concourse.tile as tile
from concourse.bass_utils import run_bass_kernel_spmd
from concourse.masks import make_identity

N_CORES = 8
B, C, H, W = 32, 128, 112, 112
BL = B // N_CORES  # images per core
HP = H + 2  # padded height/width (114)
TAPS = [(kh, kw) for kh in range(3) for kw in range(3)]

F32 = mybir.dt.float32
BF16 = mybir.dt.bfloat16

N_ROWCHUNK = 4  # output rows per PSUM accumulation group (<= one 2KB bank)
N_STAGEROWS = 28  # output rows per SBUF->DRAM store (must divide 112)
N_LOADROWS = 28  # input rows per DRAM->SBUF load
N_SIGNROWS = 14  # input rows per ACT Sign instruction


RP = 128  # fp8 padded-row pitch; 128 makes the DoubleRow mid-dim step %16==0

VARIANT = "fp8dr5"  # "bf16" | "fp8dr" | "fp8dr5"


def build_nc(variant=None):
    variant = variant or VARIANT
    fp8 = variant in ("fp8dr", "fp8dr5", "fp8dr6", "fp8dr7")
    # fp8dr5: a second, column-shifted plane P1[r,c] = P0[r,c+1] lets taps
    # (2,0)+(2,1) share one DoubleRow matmul (pair step = plane stride), so a
    # chunk needs 5 matmuls instead of 6.
    planes = variant in ("fp8dr5", "fp8dr6", "fp8dr7")
    # fp8dr6: additionally (1) leave garbage-only pad cells (whose products
    # only ever land in discarded PSUM columns) unwritten, so the first
    # matmuls don't wait on slow strided memsets; (2) alternate the P1 fill
    # between ACT Sign and a DVE shift-copy to balance engine load; (3) store
    # output in 14-row pieces to shorten the kernel tail.
    lean = variant == "fp8dr6"
    stage_rows = 16 if lean else N_STAGEROWS
    # fp8dr7: fp8dr5 scheduling, but (1) buffer-1 border memsets deferred past
    # image 0 so buffer-0 init isn't queued behind them, (2) 56-row input
    # loads for images 1..3 (better DMA efficiency; image 0 keeps 28-row loads
    # for fast pipeline fill), (3) final store split to shorten the tail.
    lean7 = variant == "fp8dr7"
    FP8 = mybir.dt.float8e4
    act_dt = FP8 if fp8 else BF16
    pitch = RP if fp8 else HP

    nc = bacc.Bacc(
        "TRN2", target_bir_lowering=False, debug=False, num_devices=N_CORES
    )
    x = nc.declare_dram_parameter("x", [BL, C, H, W], F32, isOutput=False)
    w = nc.declare_dram_parameter("weight", [C, C, 3, 3], F32, isOutput=False)
    y = nc.declare_dram_parameter("y", [BL, C, H, W], F32, isOutput=True)

    with tile.TileContext(nc) as tc:
        with (
            tc.tile_pool(name="consts", bufs=1) as consts,
            tc.tile_pool(name="psum", bufs=1, space="PSUM") as psum_pool,
        ):
            # ---- weight prep: scale[o] and transposed sign-weight tiles ----
            # bf16:  lhsT[i, tap, o] for the 9 taps
            # fp8dr: wdr[i, kw, j, o] pairs taps (kh=0,kw),(kh=1,kw); w2[i, kw, o]
            #        holds the kh=2 row
            if fp8:
                wdr = consts.tile([C, 3, 2, C], FP8)
                if planes:
                    wp2 = consts.tile([C, 2, C], FP8)  # taps (2,0),(2,1)
                    w22 = consts.tile([C, C], FP8)  # tap (2,2)
                else:
                    w2 = consts.tile([C, 3, C], FP8)
            else:
                lhsT = consts.tile([C, 9, C], BF16)  # [i, tap, o]
            scale = consts.tile([C, 1], F32)
            identity = consts.tile([C, C], BF16)
            make_identity(nc, identity)
            with tc.tile_pool(name="wprep", bufs=1) as wp:
                wf = wp.tile([C, C, 3, 3], F32)
                nc.sync.dma_start(wf[:, :, :, :], w[:, :, :, :])
                wabs = wp.tile([C, C, 3, 3], F32)
                ssum = wp.tile([C, 1], F32)
                nc.scalar.activation(
                    wabs[:, :, :, :],
                    wf[:, :, :, :],
                    mybir.ActivationFunctionType.Abs,
                    accum_out=ssum[:, :],
                )
                nc.scalar.mul(scale[:, :], ssum[:, :], 1.0 / (C * 9))
                wsign = wp.tile([C, C, 3, 3], BF16)
                nc.scalar.sign(wsign[:, :, :, :], wf[:, :, :, :])
                for t, (kh, kw) in enumerate(TAPS):
                    pst = psum_pool.tile([C, C], BF16, tag="pst", bufs=2)
                    nc.tensor.transpose(pst[:, :], wsign[:, :, kh, kw], identity[:, :])
                    if fp8 and planes:
                        if kh < 2:
                            dst = wdr[:, kw, kh, :]
                        elif kw < 2:
                            dst = wp2[:, kw, :]
                        else:
                            dst = w22[:, :]
                    elif fp8:
                        dst = wdr[:, kw, kh, :] if kh < 2 else w2[:, kw, :]
                    else:
                        dst = lhsT[:, t, :]
                    # DVE, not ACT: keeps ACT free for the first image's Sign
                    nc.vector.tensor_copy(dst, pst[:, :])

            # ---- main loop over local images ----
            with (
                tc.tile_pool(name="raw", bufs=2) as raw_pool,
                tc.tile_pool(name="xpad", bufs=1) as xpad_pool,
                tc.tile_pool(name="stage", bufs=3) as stage_pool,
            ):
                # Two persistent padded buffers, manually double-buffered
                # across images. Borders are zeroed ONCE here (the interior is
                # rewritten per image, borders stay zero), so image-boundary
                # matmuls never wait on memsets queued behind output DMAs.
                # fp8dr reads whole pitch-128 rows (N=512 contiguous spans);
                # one extra dummy row absorbs the last chunk's 2-element
                # overrun, and every non-interior cell is zeroed.
                nrows = HP + 1 if fp8 else HP
                nplanes = 2 if planes else 1

                def border_memsets(xp):
                    nc.gpsimd.memset(xp[:, 0, 0, :], 0.0)
                    nc.gpsimd.memset(xp[:, 0, HP - 1 :, :], 0.0)
                    nc.gpsimd.memset(xp[:, 0, :, W + 1 : pitch], 0.0)
                    nc.gpsimd.memset(xp[:, 0, :, 0], 0.0)
                    nc.gpsimd.memset(xp[:, 1, 0:2, :], 0.0)
                    nc.gpsimd.memset(xp[:, 1, HP - 1 :, :], 0.0)
                    nc.gpsimd.memset(xp[:, 1, :, W:pitch], 0.0)

                xpads = []
                for k in range(2):
                    xp = xpad_pool.tile(
                        [C, nplanes, nrows, pitch],
                        act_dt,
                        tag=f"xpad{k}",
                        name=f"xpad{k}",
                    )
                    xpads.append(xp)
                    if lean7:
                        if k == 0:
                            border_memsets(xp)
                        continue
                    nc.gpsimd.memset(xp[:, 0, 0, :], 0.0)
                    if lean:
                        # thin true-pad strips on gpsimd (fast), fat
                        # garbage-only strips on the (idle-at-start) DVE, so
                        # buffer init never gates the first matmuls
                        nc.gpsimd.memset(xp[:, 0, HP - 1 :, :], 0.0)
                        nc.gpsimd.memset(xp[:, 0, 1 : HP - 1, 0], 0.0)
                        nc.gpsimd.memset(xp[:, 0, 1 : HP - 1, W + 1], 0.0)
                        nc.gpsimd.memset(xp[:, 1, HP - 1 :, :], 0.0)
                        nc.vector.memset(xp[:, 0, 1 : HP - 1, W + 2 : pitch], 0.0)
                        nc.vector.memset(xp[:, 1, 2 : HP - 1, W : pitch], 0.0)
                    elif fp8:
                        nc.gpsimd.memset(xp[:, 0, HP - 1 :, :], 0.0)
                        nc.gpsimd.memset(xp[:, 0, :, W + 1 : pitch], 0.0)
                        nc.gpsimd.memset(xp[:, 0, :, 0], 0.0)
                        if planes:
                            nc.gpsimd.memset(xp[:, 1, 0:2, :], 0.0)
                            nc.gpsimd.memset(xp[:, 1, HP - 1 :, :], 0.0)
                            nc.gpsimd.memset(xp[:, 1, :, W:pitch], 0.0)
                    else:
                        nc.gpsimd.memset(xp[:, 0, HP - 1, :], 0.0)
                        nc.gpsimd.memset(xp[:, 0, :, HP - 1], 0.0)
                        nc.gpsimd.memset(xp[:, 0, :, 0], 0.0)
                for n in range(BL):
                    xim = x[n]  # [C, H, W]
                    yim = y[n]
                    xpad = xpads[n % 2]
                    if lean7 and n > 0:
                        load_sizes = [56, 56]
                    else:
                        load_sizes = [N_LOADROWS] * (H // N_LOADROWS)
                    raw_rows = 56 if lean7 else N_LOADROWS
                    r0 = 0
                    for rows in load_sizes:
                        raw = raw_pool.tile(
                            [C, raw_rows, W], F32, tag="raw",
                            bufs=2 if lean7 else 4,
                        )
                        nc.sync.dma_start(
                            raw[:, :rows, :], xim[:, r0 : r0 + rows, :]
                        )
                        for a in range(0, rows, N_SIGNROWS):
                            rr = r0 + a + 1
                            nc.scalar.sign(
                                xpad[:, 0, rr : rr + N_SIGNROWS, 1 : 1 + W],
                                raw[:, a : a + N_SIGNROWS, :],
                            )
                            if planes and lean and (a // N_SIGNROWS) % 2 == 1:
                                # balance engines: every other P1 piece is a
                                # DVE shift-copy of P0 instead of an ACT Sign
                                nc.vector.tensor_copy(
                                    xpad[:, 1, rr : rr + N_SIGNROWS, 0:W],
                                    xpad[:, 0, rr : rr + N_SIGNROWS, 1 : 1 + W],
                                )
                            elif planes:
                                nc.scalar.sign(
                                    xpad[:, 1, rr : rr + N_SIGNROWS, 0:W],
                                    raw[:, a : a + N_SIGNROWS, :],
                                )
                        r0 += rows
                    if lean7 and n == 0:
                        # buffer 1 isn't read until image 1: zero its borders
                        # only now, so buffer 0's init wasn't queued behind it
                        border_memsets(xpads[1])
                    for s0 in range(0, H, stage_rows):
                        stage = stage_pool.tile([C, stage_rows, W], F32, tag="stage")
                        for j in range(0, stage_rows, N_ROWCHUNK):
                            h0 = s0 + j
                            if fp8:
                                # full-pitch output rows: N = 4*128 = 512 fp32
                                # (one PSUM bank); cols >= 112 of each row are
                                # garbage and skipped at evacuation
                                NF = N_ROWCHUNK * pitch
                                ps = psum_pool.tile([C, NF], F32, tag="ps", bufs=6)
                                for kw in range(3):
                                    # taps (0,kw)+(1,kw) fused: K=256 DoubleRow
                                    base = xpad[:, 0, h0, kw]
                                    rhs = bass.AP(
                                        tensor=base.tensor,
                                        offset=base.offset,
                                        ap=[base.ap[0], [pitch, 2], [1, NF]],
                                    )
                                    nc.tensor.matmul(
                                        ps[:, :],
                                        wdr[:, kw, :, :],
                                        rhs,
                                        start=(kw == 0),
                                        stop=False,
                                        perf_mode=mybir.MatmulPerfMode.DoubleRow,
                                    )
                                if planes:
                                    # taps (2,0)+(2,1) fused across the P0/P1
                                    # planes (pair step = plane stride)
                                    base = xpad[:, 0, h0 + 2, 0]
                                    rhs = bass.AP(
                                        tensor=base.tensor,
                                        offset=base.offset,
                                        ap=[base.ap[0], [nrows * pitch, 2], [1, NF]],
                                    )
                                    nc.tensor.matmul(
                                        ps[:, :],
                                        wp2[:, :, :],
                                        rhs,
                                        start=False,
                                        stop=False,
                                        perf_mode=mybir.MatmulPerfMode.DoubleRow,
                                    )
                                    base = xpad[:, 0, h0 + 2, 2]
                                    rhs = bass.AP(
                                        tensor=base.tensor,
                                        offset=base.offset,
                                        ap=[base.ap[0], [1, NF]],
                                    )
                                    nc.tensor.matmul(
                                        ps[:, :],
                                        w22[:, :],
                                        rhs,
                                        start=False,
                                        stop=True,
                                    )
                                else:
                                    for kw in range(3):
                                        # tap (2,kw)
                                        base = xpad[:, 0, h0 + 2, kw]
                                        rhs = bass.AP(
                                            tensor=base.tensor,
                                            offset=base.offset,
                                            ap=[base.ap[0], [1, NF]],
                                        )
                                        nc.tensor.matmul(
                                            ps[:, :],
                                            w2[:, kw, :],
                                            rhs,
                                            start=False,
                                            stop=(kw == 2),
                                        )
                                ps_rows = ps.rearrange(
                                    "p (a b) -> p a b", b=pitch
                                )[:, :, 0:W]
                            else:
                                ps = psum_pool.tile(
                                    [C, N_ROWCHUNK, W], F32, tag="ps", bufs=6
                                )
                                for t, (kh, kw) in enumerate(TAPS):
                                    nc.tensor.matmul(
                                        ps[:, :, :],
                                        lhsT[:, t, :],
                                        xpad[
                                            :,
                                            0,
                                            h0 + kh : h0 + kh + N_ROWCHUNK,
                                            kw : kw + W,
                                        ],
                                        start=(t == 0),
                                        stop=(t == len(TAPS) - 1),
                                    )
                                ps_rows = ps[:, :, :]
                            nc.vector.tensor_scalar_mul(
                                stage[:, j : j + N_ROWCHUNK, :], ps_rows, scale[:, :]
                            )
                        if lean7 and n == BL - 1 and s0 == H - stage_rows:
                            # split the very last store so the kernel tail only
                            # waits on half the bytes
                            hs = stage_rows // 2
                            nc.gpsimd.dma_start(
                                yim[:, s0 : s0 + hs, :], stage[:, :hs, :]
                            )
                            nc.gpsimd.dma_start(
                                yim[:, s0 + hs : s0 + stage_rows, :],
                                stage[:, hs:, :],
                            )
                        else:
                            nc.gpsimd.dma_start(
                                yim[:, s0 : s0 + stage_rows, :], stage[:, :, :]
                            )

    nc.compile()
    return nc


_NC_CACHE = {}


def _get_nc(variant=None):
    variant = variant or VARIANT
    if variant not in _NC_CACHE:
        _NC_CACHE[variant] = build_nc(variant)
    return _NC_CACHE[variant]


def kernel(
    x: np.ndarray,
    weight: np.ndarray,
    _trace: bool = False,
    _variant: str | None = None,
    **_kw,
):
    assert x.shape == (B, C, H, W) and weight.shape == (C, C, 3, 3)
    nc = _get_nc(_variant)
    xs = np.ascontiguousarray(x, dtype=np.float32)
    wgt = np.ascontiguousarray(weight, dtype=np.float32)
    in_maps = [
        {"x": xs[i * BL : (i + 1) * BL], "weight": wgt} for i in range(N_CORES)
    ]
    res = run_bass_kernel_spmd(
        nc, in_maps, core_ids=list(range(N_CORES)), trace=_trace
    )
    out = np.concatenate([res.results[i]["y"] for i in range(N_CORES)], axis=0)
    if _trace:
        kernel.last_results = res
    return out
